# revision 14
# baseline (speedup 1.0000x reference)
"""Trainium2 Bass kernel for nn_Net_63754494542044 (v2).

Data-parallel over 8 NeuronCores (8 B-samples each). Host pre-packs
conv1 im2col / conv weights / RoIAlign-as-matmul weights / fused GNN
weights; device runs conv1 -> conv2 -> pool-matmul -> fc0/emb/red ->
8 fused GNN rollouts. No DRAM round-trip, no indirect DMA.
"""
import sys
sys.path.insert(0, '/opt/trn_rl_repo')
import numpy as np
from contextlib import ExitStack
import concourse.bass as bass
import concourse.tile as tile
from concourse import mybir
from concourse.bass_utils import run_bass_kernel_spmd

# Walrus wait-slot limits: CTRL-encoded (Drain/NoOp) = 1; others appear
# limited too on this build -- split conservatively.
def split_drain_waits(nc, max_waits=1, max_waits_other=1):
    for fn in nc.m.functions:
        for bb in fn.blocks:
            insts = bb.instructions
            i = 0
            while i < len(insts):
                inst = insts[i]
                si = getattr(inst, 'sync_info', None)
                lim = max_waits if isinstance(inst, (mybir.InstDrain, mybir.InstNoOp)) else max_waits_other
                if si is not None and si.on_wait and len(si.on_wait) > lim:
                    waits = list(si.on_wait)
                    keep = waits[-lim:]
                    extra = waits[:-lim]
                    new_nops = []
                    for k in range(0, len(extra), max_waits):
                        chunk = extra[k:k + max_waits]
                        nop = mybir.InstNoOp(
                            name=nc.get_next_instruction_name(),
                            engine=inst.engine,
                        )
                        nop.sync_info = mybir.SyncInfo(on_wait=chunk, on_update=[])
                        nc.register_instruction(nop)
                        new_nops.append(nop)
                    inst.sync_info = mybir.SyncInfo(on_wait=keep, on_update=list(si.on_update))
                    insts[i:i] = new_nops
                    i += len(new_nops)
                i += 1


B, T, N = 64, 4, 6
IMG, CIN = 128, 3
VE, D, P = 64, 256, 4
SCALE = 0.25
NCORE = 8
BC = B // NCORE          # 8 samples per core
NIMG = BC * T            # 32 images per core
NROI = BC * T * N        # 192 rois per core
NROW = BC * N            # 48 gnn rows per core
NPT = NROI * 16          # 3072 sample points per core


# ---------------- conv1 im2col (host) ----------------
# conv1: 3->64, 3x3, stride2, SAME on 128x128 -> 64x64.
# 2-px-packed output: out pair (oy, j) covers ox = 2j, 2j+1.
# K=45 rows: (rowtap rt in 0..2) x (coltap ct in 0..4) x (ci in 0..2)
def conv1_im2col_host(x):  # x [nimg, 3, 128, 128] fp32
    nimg = x.shape[0]
    xp = np.pad(x, ((0, 0), (0, 0), (0, 1), (0, 1)))  # SAME stride2: pad bottom/right only
    cols = np.empty((45, nimg, 64, 32), np.float32)
    k = 0
    for rt in range(3):
        for ct in range(5):
            for ci in range(3):
                cols[k] = xp[:, ci, rt:rt + 127:2, ct:ct + 125:4]
                k += 1
    return cols  # [45, nimg, 64, 32]


def conv1_weights_host(w_conv1):  # [64, 3, 3, 3]
    W2 = np.zeros((45, 128), np.float32)
    for px in range(2):
        for oc in range(64):
            m = px * 64 + oc
            for dy in range(3):
                for dx in range(3):
                    ct = 2 * px + dx
                    for ci in range(3):
                        W2[(dy * 5 + ct) * 3 + ci, m] = w_conv1[oc, ci, dy, dx]
    return W2


# ---------------- conv2 weights (host) ----------------
def conv2_weights_host(w_conv2):  # [64, 64, 3, 3]
    Wb = [[np.zeros((128, 128), np.float32) for _ in range(2)] + [np.zeros((64, 128), np.float32)]
          for _ in range(3)]
    for pxo in range(2):
        for oc in range(64):
            m = pxo * 64 + oc
            for dy in range(3):
                for dx in range(3):
                    x_off = 2 * pxo + dx
                    pxi = x_off % 2
                    Xrel = x_off // 2
                    for ci in range(64):
                        if Xrel < 2:
                            Wb[dy][Xrel][pxi * 64 + ci, m] += w_conv2[oc, ci, dy, dx]
                        else:
                            Wb[dy][2][ci, m] += w_conv2[oc, ci, dy, dx]
    return Wb


# ---------------- RoIAlign-as-matmul weights (host) ----------------
# pooledT_img [64 c, 96 pt] = sum_{c8,p} f2t_slice[c8][p, c] * Wsp[img][c8, p, pt]
# f2t slice (c8=(b,pxo)): partitions p = (y,j2)-lin b*128+p, channel c; x=2*j2+pxo.
_PIX_Y = None


def wsp_host(rois):
    """rois [NROI,5] (local img idx in col 0). Returns [128, NIMG*8*96] bf-ready f32."""
    W = H = 32
    nroi = rois.shape[0]
    x1 = rois[:, 1] * SCALE; y1 = rois[:, 2] * SCALE
    x2 = rois[:, 3] * SCALE; y2 = rois[:, 4] * SCALE
    bw = np.maximum(x2 - x1, 1.0) / P
    bh = np.maximum(y2 - y1, 1.0) / P
    grid = np.arange(P, dtype=np.float32) + 0.5
    sx = np.broadcast_to(x1[:, None, None] + bw[:, None, None] * grid[None, None, :], (nroi, P, P)).reshape(-1)
    sy = np.broadcast_to(y1[:, None, None] + bh[:, None, None] * grid[None, :, None], (nroi, P, P)).reshape(-1)
    x0f = np.clip(np.floor(sx), 0, W - 1)
    y0f = np.clip(np.floor(sy), 0, H - 1)
    lx = np.clip(sx - x0f, 0.0, 1.0).astype(np.float32)
    ly = np.clip(sy - y0f, 0.0, 1.0).astype(np.float32)
    x0 = x0f.astype(np.int64); y0 = y0f.astype(np.int64)
    hi = x0 >= 31
    x0 = np.where(hi, 30, x0); lx = np.where(hi, 1.0, lx)
    hiy = y0 >= 31
    y0 = np.where(hiy, 30, y0); ly = np.where(hiy, 1.0, ly)
    npt = nroi * 16
    img = np.arange(npt) // 96          # 6 rois * 16 pts per img
    ptl = np.arange(npt) % 96           # point index within img
    Wimg = np.zeros((NIMG, 1024, 96), np.float32)
    for dy, wy in ((0, 1 - ly), (1, ly)):
        for dx, wx in ((0, 1 - lx), (1, lx)):
            pix = (y0 + dy) * 32 + (x0 + dx)
            np.add.at(Wimg, (img, pix, ptl), wy * wx)
    # reorder pix -> (c8, p):  c8=(b,pxo), p: (y,j2)=b*128+p, x=2*j2+pxo
    c8 = np.arange(8); b = c8 // 2; pxo = c8 % 2
    p = np.arange(128)
    lin = b[:, None] * 128 + p[None, :]           # [8,128] (y,j2) linear
    y = lin // 16; j2 = lin % 16
    pix_perm = y * 32 + 2 * j2 + pxo[:, None]     # [8,128]
    Wr = Wimg[:, pix_perm, :]                     # [NIMG, 8, 128, 96]
    return np.ascontiguousarray(Wr.transpose(2, 0, 1, 3)).reshape(128, NIMG * 8 * 96)


# ---------------- device input packing ----------------
def make_core_inputs(inputs, shard):
    import ml_dtypes
    bf16 = ml_dtypes.bfloat16
    sl = slice(shard * BC, (shard + 1) * BC)
    x = np.asarray(inputs['x'][sl], np.float32).reshape(NIMG, CIN, IMG, IMG)
    rois = np.asarray(inputs['rois'][sl], np.float32).reshape(NROI, 5)
    coor = np.asarray(inputs['src_coor_features'][sl], np.float32)   # [BC,T,N,2]
    rr5 = rois.reshape(BC, T, N, 5)
    r = (((rr5[..., 4] - rr5[..., 2]) / 2 + (rr5[..., 3] - rr5[..., 1]) / 2) / 2).mean(1)

    d = {}
    cols = conv1_im2col_host(x)                       # [45, NIMG, 64, 32]
    # 2-half layout: [NGRP, 109, 8192]: rows 0-44 = taps imgs 0-3 of group,
    # rows 64-108 = imgs 4-7; rows 45-63 zero pad.
    cols = cols.reshape(45, 4, 2, 4, 2048)            # [45, grp, half, img4, 2048]
    imc = np.zeros((4, 109, 8192), np.float32)
    imc[:, 0:45] = cols[:, :, 0].reshape(45, 4, 8192).transpose(1, 0, 2)
    imc[:, 64:109] = cols[:, :, 1].reshape(45, 4, 8192).transpose(1, 0, 2)
    d['im2col'] = imc.astype(bf16)
    W2 = conv1_weights_host(np.asarray(inputs['w_conv1']))
    w1d = np.zeros((109, 128), np.float32)
    w1d[0:45] = W2; w1d[64:109] = W2
    d['w1'] = w1d.astype(bf16)
    b1 = np.asarray(inputs['b_conv1'], np.float32)
    d['b1'] = np.tile(b1, 2).reshape(128, 1).astype(np.float32)
    Wb = conv2_weights_host(np.asarray(inputs['w_conv2']))
    d['w2a'] = np.stack([Wb[dy][0] for dy in range(3)]).astype(bf16)
    d['w2b'] = np.stack([Wb[dy][1] for dy in range(3)]).astype(bf16)
    d['w2c'] = np.stack([Wb[dy][2] for dy in range(3)]).astype(bf16)
    b2 = np.asarray(inputs['b_conv2'], np.float32)
    d['b2'] = np.tile(b2, 2).reshape(128, 1).astype(np.float32)

    d['wsp'] = wsp_host(rois).astype(bf16)            # [128, 24576]

    fc0w = np.asarray(inputs['fc0_w'], np.float32).reshape(D, 64, 16)  # [d, c, pt]
    d['fc0t'] = np.ascontiguousarray(fc0w.transpose(2, 1, 0)).astype(bf16)  # [pt, c, d]
    d['fc0b'] = np.asarray(inputs['fc0_b'], np.float32).reshape(2, 128).T.copy()

    d['coor_fm'] = coor.reshape(NROI, 2).T.astype(bf16).copy()

    def t2(w):   # [256, K] -> [kc, 128, 256] lhsT chunks (w.T row-chunks)
        wT = np.ascontiguousarray(np.asarray(w, np.float32).T)       # [K, 256]
        K = wT.shape[0]
        return wT.reshape(K // 128, 128, 256)

    def bcol(b):  # [256] -> [128, 2]
        return np.asarray(b, np.float32).reshape(2, 128).T.copy()

    d['fc0ct'] = np.asarray(inputs['fc0c_w'], np.float32).T.astype(bf16).copy()  # [2, 256]
    d['fc0cb'] = bcol(inputs['fc0c_b'])
    d['fc1ct'] = t2(inputs['fc1c_w']).astype(bf16)
    d['fc1cb'] = bcol(inputs['fc1c_b'])
    redw = np.asarray(inputs['red_w'], np.float32)
    d['redoT'] = t2(redw[:, :D]).astype(bf16)
    d['redeT'] = t2(redw[:, D:]).astype(bf16)
    d['redb'] = bcol(inputs['red_b'])

    # ---- fused GNN weights ----
    aw = np.asarray(inputs['g_aff_w'], np.float32)    # [4, D, D]
    sw = np.asarray(inputs['g_self_w'], np.float32)
    grw = np.asarray(inputs['g_rel_w'], np.float32)   # [4, D, 2D]
    gow = np.asarray(inputs['g_out_w'], np.float32)
    sb = np.asarray(inputs['g_self_b'], np.float32)   # [4, D]
    rb = np.asarray(inputs['g_rel_b'], np.float32)
    ab = np.asarray(inputs['g_aff_b'], np.float32)
    ob = np.asarray(inputs['g_out_b'], np.float32)
    A1 = np.stack([aw[k] @ sw[k] for k in range(4)])            # x-term on s
    A2 = np.stack([aw[k] @ grw[k][:, :D] for k in range(4)])    # on s*deg
    A3 = np.stack([aw[k] @ grw[k][:, D:] for k in range(4)])    # v-term
    rbA = np.stack([rb[k] @ aw[k].T for k in range(4)])         # [4, D]
    cA = np.stack([sb[k] @ aw[k].T + ab[k] for k in range(4)])  # [4, D]
    d['A1T'] = np.stack([t2(A1[k]) for k in range(4)]).astype(bf16)
    d['A2T'] = np.stack([t2(A2[k]) for k in range(4)]).astype(bf16)
    d['A3T'] = np.stack([t2(A3[k]) for k in range(4)]).astype(bf16)
    d['gowaT'] = np.stack([t2(gow[k][:, :D]) for k in range(4)]).astype(bf16)
    d['gowsT'] = np.stack([t2(gow[k][:, D:]) for k in range(4)]).astype(bf16)
    # v_ext rows 48/49: [2, 1024] col = k*256 + d
    d['vext_init'] = np.stack([rbA.reshape(-1), cA.reshape(-1)]).astype(bf16)
    # gob row: col = (k*2+m2)*128 + dl
    d['gob_row'] = ob.reshape(1, -1).astype(bf16)
    d['aggT'] = t2(inputs['agg_w']).astype(bf16)                # [8, 128, 256]
    d['aggb'] = bcol(inputs['agg_b'])
    decw = np.asarray(inputs['dec_w'], np.float32)              # [4, 256]
    d['decT'] = decw.T.reshape(2, 128, 4).astype(bf16).copy()
    d['decb'] = np.asarray(inputs['dec_b'], np.float32).reshape(4, 1).copy()
    d['decb2'] = np.asarray(inputs['dec_b'], np.float32)[2:4].reshape(2, 1).copy()

    # masks: hm_ext [4, 50, 48] (rows 0:48 mask, 48 deg, 49 ones); hdeg [4, 128, 96]
    hms, hds = [], []
    for m in range(4):
        bm = np.zeros((NROW, NROW), np.float32)
        cm = coor[:, m]
        for b in range(BC):
            dist = np.linalg.norm(cm[b][:, None, :] - cm[b][None, :, :], axis=-1)
            msk = (dist <= (r[b][:, None] + r[b][None, :])) & ~np.eye(N, dtype=bool)
            bm[b * N:(b + 1) * N, b * N:(b + 1) * N] = msk
        deg = bm.sum(1)
        ext = np.zeros((66, NROW), np.float32)
        ext[0:48] = bm; ext[64] = deg; ext[65] = 1.0
        hms.append(ext)
        hds.append(np.broadcast_to(deg[None, :], (128, NROW)))
    d['hm'] = np.stack(hms).astype(bf16)
    d['hdeg'] = np.ascontiguousarray(
        np.concatenate([np.stack(hds)] * 2, axis=2)).astype(np.float32)  # [4,128,96]
    Tmat = np.full((NROW, NROW), -1.0, np.float32)
    for b in range(BC):
        rs = (r[b][:, None] + r[b][None, :]) ** 2
        np.fill_diagonal(rs, -1.0)
        Tmat[b * N:(b + 1) * N, b * N:(b + 1) * N] = rs
    d['Tm'] = Tmat
    d['ones48'] = np.ones((48, 128), bf16)
    d['ones2'] = np.ones((2, 48), bf16)
    d['zpad'] = np.zeros((16, 1024), bf16)
    d['ident'] = np.eye(128, dtype=bf16)
    return d


dt = mybir.dt
AF = mybir.ActivationFunctionType
OP = mybir.AluOpType

IMG_GRP = 8        # images per conv group
NGRP = NIMG // IMG_GRP
IMGF = 2 * 33 * 33  # 2178 free els per img in feat1_ph


def build(nc: bass.Bass, stage="full"):
    f32, bf16, i32 = dt.float32, dt.bfloat16, dt.int32

    def din(name, shape, d):
        return nc.dram_tensor(name, shape, d, kind="ExternalInput")

    im2col = din("im2col", [4, 109, 8192], bf16)
    w1 = din("w1", [109, 128], bf16)
    b1 = din("b1", [128, 1], f32)
    w2a = din("w2a", [3, 128, 128], bf16)
    w2b = din("w2b", [3, 128, 128], bf16)
    w2c = din("w2c", [3, 64, 128], bf16)
    b2 = din("b2", [128, 1], f32)
    wsp = din("wsp", [128, 24576], bf16)
    fc0t = din("fc0t", [16, 64, 256], bf16)
    fc0b = din("fc0b", [128, 2], f32)
    coor = din("coor_fm", [2, 192], bf16)
    fc0ct = din("fc0ct", [2, 256], bf16)
    fc0cb = din("fc0cb", [128, 2], f32)
    fc1ct = din("fc1ct", [2, 128, 256], bf16)
    fc1cb = din("fc1cb", [128, 2], f32)
    redoT = din("redoT", [2, 128, 256], bf16)
    redeT = din("redeT", [2, 128, 256], bf16)
    redb = din("redb", [128, 2], f32)
    A1T = din("A1T", [4, 2, 128, 256], bf16)
    A2T = din("A2T", [4, 2, 128, 256], bf16)
    A3T = din("A3T", [4, 2, 128, 256], bf16)
    gowaT = din("gowaT", [4, 2, 128, 256], bf16)
    gowsT = din("gowsT", [4, 2, 128, 256], bf16)
    vext_init = din("vext_init", [2, 1024], bf16)
    gob_row = din("gob_row", [1, 1024], bf16)
    aggT = din("aggT", [8, 128, 256], bf16)
    aggb = din("aggb", [128, 2], f32)
    decT = din("decT", [2, 128, 4], bf16)
    decb = din("decb", [4, 1], f32)
    decb2 = din("decb2", [2, 1], f32)
    hm = din("hm", [4, 66, 48], bf16)
    hdeg = din("hdeg", [4, 128, 96], f32)
    Tm = din("Tm", [48, 48], f32)
    ones48 = din("ones48", [48, 128], bf16)
    ones2 = din("ones2", [2, 48], bf16)
    zpad = din("zpad", [16, 1024], bf16)
    ident = din("ident", [128, 128], bf16)

    out = nc.dram_tensor("bbox_out", [8, 8, 6, 4], f32, kind="ExternalOutput")

    with tile.TileContext(nc) as tc, ExitStack() as ctx:
        # ---- persistent pools ----
        wp = ctx.enter_context(tc.tile_pool(name="w", bufs=1))
        sp = ctx.enter_context(tc.tile_pool(name="state", bufs=1))

        def load(dram_t, shape, dtype, src_ap=None):
            t = wp.tile(shape, dtype, tag=dram_t.name)
            if src_ap is None:
                nc.sync.dma_start(t[:], dram_t[:, :])
            else:
                dims = [c for _, c in src_ap.ap[1:]]
                spec = " ".join(f"d{i}" for i in range(len(dims)))
                kw = {f"d{i}": dims[i] for i in range(len(dims) - 1)}
                dv = t[:].rearrange(f"p ({spec}) -> p {spec}", **kw)
                nc.sync.dma_start(dv, src_ap)
            return t

        # conv-critical weights first (DMA priority = emission order)
        w1_s = load(w1, [109, 128], bf16)
        b1_s = load(b1, [128, 1], f32)
        w2a_s = load(w2a, [128, 3 * 128], bf16, w2a[:].rearrange("d p m -> p d m"))
        w2b_s = load(w2b, [128, 3 * 128], bf16, w2b[:].rearrange("d p m -> p d m"))
        w2c_s = load(w2c, [64, 3 * 128], bf16, w2c[:].rearrange("d p m -> p d m"))
        b2_s = load(b2, [128, 1], f32)
        ident_s = load(ident, [128, 128], bf16)

        st = [sp.tile([128, 96], bf16, name=f"st{m}", tag=f"st{m}") for m in range(12)]
        bbox_sb = sp.tile([4, 384], f32, tag="bbox")
        poolT = sp.tile([64, 3072], bf16, tag="poolT")

        if stage == "dma":
            nc.gpsimd.memset(bbox_sb[:], 0.0)
            nc.sync.dma_start(out[:].rearrange("b rr n f -> f (b rr n)"), bbox_sb[:])
            return nc

        # ================= conv + pool stage =================
        with ExitStack() as cvx, nc.named_scope("conv"):
            imcp = cvx.enter_context(tc.tile_pool(name="imc", bufs=2))
            wspp = cvx.enter_context(tc.tile_pool(name="wspp", bufs=2))
            f1p = cvx.enter_context(tc.tile_pool(name="f1", bufs=2))
            c1ps = cvx.enter_context(tc.tile_pool(name="c1ps", bufs=2, space="PSUM"))
            c2ps = cvx.enter_context(tc.tile_pool(name="c2ps", bufs=2, space="PSUM"))
            tps = cvx.enter_context(tc.tile_pool(name="tps", bufs=1, space="PSUM"))
            pps = cvx.enter_context(tc.tile_pool(name="pps", bufs=1, space="PSUM"))
            f2p = cvx.enter_context(tc.tile_pool(name="f2", bufs=3))

            for g in range(NGRP):
                imc = imcp.tile([109, 8192], bf16, tag="imc")
                nc.sync.dma_start(imc[:], im2col[g])
                wsp_t = wspp.tile([128, 6144], bf16, tag="wsp")
                nc.sync.dma_start(wsp_t[:], wsp[:, g * 6144:(g + 1) * 6144])
                f1 = f1p.tile([128, IMG_GRP * IMGF], bf16, tag="f1")
                # zero halo strips (Y=32 row, X=32 col)
                f1v = f1[:].rearrange("p (i y x) -> p i y x", i=IMG_GRP, y=2 * 33, x=33)
                nc.gpsimd.memset(f1v[:, :, :, 32:33], 0.0)
                f1h = f1[:].rearrange("p (i py y x) -> p i py y x", i=IMG_GRP, py=2, y=33, x=33)
                nc.gpsimd.memset(f1h[:, :, :, 32:33, :], 0.0)
                for i in range(IMG_GRP):
                    hb2 = (i // 4) * 64          # partition base (imgs 0-3 vs 4-7)
                    io = (i % 4) * 2048
                    # conv1: 4 matmuls of [45,128]x[45,512] -> psum [128,1024] x2
                    pv = []
                    for h in range(2):
                        ps = c1ps.tile([128, 1024], f32, tag="c1")
                        for q in range(2):
                            nc.tensor.matmul(ps[:, q * 512:(q + 1) * 512],
                                             lhsT=w1_s[hb2:hb2 + 45, :],
                                             rhs=imc[hb2:hb2 + 45,
                                                     io + h * 1024 + q * 512:
                                                     io + h * 1024 + (q + 1) * 512],
                                             start=True, stop=True)
                        pv.append(ps)
                    for h in range(2):
                        psv = pv[h][:].rearrange("p (y j) -> p y j", y=32)
                        for py in range(2):
                            dst = f1h[:, i, py, 16 * h:16 * h + 16, 0:32]
                            if py:
                                nc.vector.tensor_scalar(
                                    out=dst, in0=psv[:, py::2, :], scalar1=b1_s[:, 0:1],
                                    scalar2=0.0, op0=OP.add, op1=OP.max)
                            else:
                                nc.scalar.activation(out=dst, in_=psv[:, py::2, :],
                                                     func=AF.Relu, bias=b1_s[:, 0:1])
                for i in range(IMG_GRP):
                    if stage == "conv1":
                        continue
                    # conv2: 9 matmuls -> psum [128, 512] cols (oy2 32, j2 16)
                    ps = c2ps.tile([128, 512], f32, tag="c2")
                    first = True
                    f1v5 = f1[:].rearrange("p (i py y x) -> p i py y x",
                                           i=IMG_GRP, py=2, y=33, x=33)
                    for dy in range(3):
                        py, yo = dy % 2, dy // 2
                        for gsel in range(3):
                            sl = f1v5[:, i, py, yo:yo + 32, gsel:gsel + 1]
                            rhs_ap = bass.AP(sl.tensor, sl.offset,
                                             [sl.ap[0], sl.ap[1], [2, 16]])
                            if gsel == 2:
                                rhs_ap = rhs_ap[0:64]
                                lhsT = w2c_s[:, dy * 128:(dy + 1) * 128]
                            else:
                                lhsT = (w2a_s if gsel == 0 else w2b_s)[:, dy * 128:(dy + 1) * 128]
                            nc.tensor.matmul(ps[:], lhsT=lhsT, rhs=rhs_ap,
                                             start=first, stop=(dy == 2 and gsel == 2))
                            first = False
                    f2s = f2p.tile([128, 512], bf16, tag="f2s")
                    if i % 2 == 0:
                        nc.vector.tensor_scalar(out=f2s[:], in0=ps[:], scalar1=b2_s[:, 0:1],
                                                scalar2=0.0, op0=OP.add, op1=OP.max)
                    else:
                        nc.scalar.activation(out=f2s[:], in_=ps[:], func=AF.Relu,
                                             bias=b2_s[:, 0:1])
                    tp = tps.tile([128, 512], bf16, tag="tp")
                    for b in range(4):
                        nc.tensor.transpose(tp[:, b * 128:(b + 1) * 128],
                                            f2s[:, b * 128:(b + 1) * 128], ident_s[:])
                    f2t = f2p.tile([128, 512], bf16, tag="f2t")
                    if i % 2 == 0:
                        nc.scalar.activation(out=f2t[:], in_=tp[:], func=AF.Copy)
                    else:
                        nc.vector.tensor_copy(out=f2t[:], in_=tp[:])
                    # pool matmuls: pooledT [64, 96] = sum_c8 f2t_slice.T @ wsp_slice
                    if stage == "conv2ns":
                        continue
                    pp = pps.tile([64, 96], f32, tag="pool")
                    for c8 in range(8):
                        nc.tensor.matmul(pp[:], lhsT=f2t[:, c8 * 64:(c8 + 1) * 64],
                                         rhs=wsp_t[:, i * 768 + c8 * 96:i * 768 + (c8 + 1) * 96],
                                         start=(c8 == 0), stop=(c8 == 7))
                    img = g * IMG_GRP + i
                    if img % 2 == 0:
                        nc.scalar.activation(out=poolT[:, img * 96:(img + 1) * 96],
                                             in_=pp[:], func=AF.Copy)
                    else:
                        nc.vector.tensor_copy(out=poolT[:, img * 96:(img + 1) * 96], in_=pp[:])

        if stage in ("conv", "conv1", "conv2ns"):
            nc.gpsimd.memset(bbox_sb[:], 0.0)
            nc.sync.dma_start(out[:].rearrange("b rr n f -> f (b rr n)"), bbox_sb[:])
            return nc

        # ---- remaining weight loads (lower DMA priority than conv) ----
        fc0t_s = load(fc0t, [64, 16 * 256], bf16, fc0t[:].rearrange("t p m -> p t m"))
        fc0b_s = load(fc0b, [128, 2], f32)
        coor_s = load(coor, [2, 192], bf16)
        fc0ct_s = load(fc0ct, [2, 256], bf16)
        fc0cb_s = load(fc0cb, [128, 2], f32)
        fc1ct_s = load(fc1ct, [128, 512], bf16, fc1ct[:].rearrange("k p m -> p k m"))
        fc1cb_s = load(fc1cb, [128, 2], f32)
        redoT_s = load(redoT, [128, 512], bf16, redoT[:].rearrange("k p m -> p k m"))
        redeT_s = load(redeT, [128, 512], bf16, redeT[:].rearrange("k p m -> p k m"))
        redb_s = load(redb, [128, 2], f32)

        def loadg(t):  # [4,2,128,256] -> [128, 4*512]
            return load(t, [128, 2048], bf16, t[:].rearrange("h k p m -> p h k m"))
        A1T_s, A2T_s, A3T_s = loadg(A1T), loadg(A2T), loadg(A3T)
        gowaT_s, gowsT_s = loadg(gowaT), loadg(gowsT)
        gob_s = load(gob_row, [1, 1024], bf16)
        aggT_s = load(aggT, [128, 2048], bf16, aggT[:].rearrange("k p m -> p k m"))
        aggb_s = load(aggb, [128, 2], f32)
        decT_s = load(decT, [128, 8], bf16, decT[:].rearrange("k p m -> p k m"))
        decb_s = load(decb, [4, 1], f32)
        decb2_s = load(decb2, [2, 1], f32)
        Tm_s = load(Tm, [48, 48], f32)
        ones48_s = load(ones48, [48, 128], bf16)
        ones2_s = load(ones2, [2, 48], bf16)

        # mask/deg slots (11 = 4 host + 7 device-computed)
        mask_t = [sp.tile([66, 48], bf16, name=f"mask{m}", tag=f"mask{m}") for m in range(11)]
        deg_t = [sp.tile([128, 96], f32, name=f"deg{m}", tag=f"deg{m}") for m in range(11)]
        for m in range(4):
            nc.sync.dma_start(mask_t[m][:], hm[m])
            nc.sync.dma_start(deg_t[m][:], hdeg[m])
        for m in range(4, 11):
            nc.sync.dma_start(mask_t[m][48:64, :], zpad[:, 0:48])
            nc.sync.dma_start(mask_t[m][65:66, :], ones2[0:1, :])
        v_ext = sp.tile([66, 1024], bf16, tag="vext")
        nc.sync.dma_start(v_ext[48:64, :], zpad[:, :])
        nc.sync.dma_start(v_ext[64:66, :], vext_init[:, :])

        # ================= fc0 + emb + red =================
        with ExitStack() as gx, nc.named_scope("fc"):
            ops = gx.enter_context(tc.tile_pool(name="ops", bufs=2, space="PSUM"))

            obj = sp.tile([128, 384], bf16, tag="obj")
            pview = poolT[:].rearrange("p (r t) -> p t r", t=16)
            for m2 in range(2):
                ps = ops.tile([128, 192], f32, tag="obj")
                for pt_i in range(16):
                    nc.tensor.matmul(ps[:], lhsT=fc0t_s[:, pt_i * 256 + m2 * 128:
                                                        pt_i * 256 + m2 * 128 + 128],
                                     rhs=pview[:, pt_i, :],
                                     start=(pt_i == 0), stop=(pt_i == 15))
                nc.scalar.activation(out=obj[:, m2 * 192:(m2 + 1) * 192], in_=ps[:],
                                     func=AF.Relu, bias=fc0b_s[:, m2:m2 + 1])
            emb1 = sp.tile([128, 384], bf16, tag="emb1")
            for m2 in range(2):
                ps = ops.tile([128, 192], f32, tag="emb")
                nc.tensor.matmul(ps[:], lhsT=fc0ct_s[:, m2 * 128:(m2 + 1) * 128],
                                 rhs=coor_s[:], start=True, stop=True)
                nc.scalar.activation(out=emb1[:, m2 * 192:(m2 + 1) * 192], in_=ps[:],
                                     func=AF.Relu, bias=fc0cb_s[:, m2:m2 + 1])
            emb2 = sp.tile([128, 384], bf16, tag="emb2")
            for m2 in range(2):
                ps = ops.tile([128, 192], f32, tag="emb")
                for kc in range(2):
                    nc.tensor.matmul(ps[:], lhsT=fc1ct_s[:, kc * 256 + m2 * 128:
                                                         kc * 256 + m2 * 128 + 128],
                                     rhs=emb1[:, kc * 192:(kc + 1) * 192],
                                     start=(kc == 0), stop=(kc == 1))
                nc.scalar.activation(out=emb2[:, m2 * 192:(m2 + 1) * 192], in_=ps[:],
                                     func=AF.Relu, bias=fc1cb_s[:, m2:m2 + 1])
            o2 = sp.tile([128, 384], bf16, tag="o2")
            for m2 in range(2):
                ps = ops.tile([128, 192], f32, tag="o2")
                for kc in range(2):
                    nc.tensor.matmul(ps[:], lhsT=redoT_s[:, kc * 256 + m2 * 128:
                                                         kc * 256 + m2 * 128 + 128],
                                     rhs=obj[:, kc * 192:(kc + 1) * 192],
                                     start=(kc == 0), stop=False)
                for kc in range(2):
                    nc.tensor.matmul(ps[:], lhsT=redeT_s[:, kc * 256 + m2 * 128:
                                                         kc * 256 + m2 * 128 + 128],
                                     rhs=emb2[:, kc * 192:(kc + 1) * 192],
                                     start=False, stop=(kc == 1))
                nc.scalar.activation(out=o2[:, m2 * 192:(m2 + 1) * 192], in_=ps[:],
                                     func=AF.Relu, bias=redb_s[:, m2:m2 + 1])
            # initial states: s_m [128, 96] cols m2*48 + b*6 + n
            o2v = o2[:].rearrange("p (m2 b t n) -> p m2 b t n", m2=2, b=8, t=4)
            for m in range(4):
                nc.vector.tensor_copy(
                    out=st[m][:].rearrange("p (m2 b n) -> p m2 b n", m2=2, b=8),
                    in_=o2v[:, :, :, m, :])

        if stage == "fc":
            nc.gpsimd.memset(bbox_sb[:], 0.0)
            nc.sync.dma_start(out[:].rearrange("b rr n f -> f (b rr n)"), bbox_sb[:])
            return nc

        # ================= GNN rollouts (fused) =================
        with ExitStack() as rx, nc.named_scope("gnn"):
            gps = rx.enter_context(tc.tile_pool(name="gps", bufs=2, space="PSUM"))
            vps = rx.enter_context(tc.tile_pool(name="vps", bufs=1, space="PSUM"))
            sps = rx.enter_context(tc.tile_pool(name="sps", bufs=2, space="PSUM"))
            hb = rx.enter_context(tc.tile_pool(name="hbuf", bufs=3))

            for rr in range(8):
                sdeg = []
                for k in range(4):
                    sd = hb.tile([128, 96], bf16, tag=f"sdeg{k}")
                    nc.vector.tensor_tensor(out=sd[:], in0=st[rr + k][:],
                                            in1=deg_t[rr + k][:], op=OP.mult)
                    sdeg.append(sd)
                v_ps = vps.tile([48, 1024], f32, tag="v")
                for k in range(4):
                    for kc in range(2):
                        nc.tensor.matmul(v_ps[:, k * 256:(k + 1) * 256],
                                         lhsT=st[rr + k][:, kc * 48:kc * 48 + 48],
                                         rhs=A3T_s[:, k * 512 + kc * 256:
                                                   k * 512 + (kc + 1) * 256],
                                         start=(kc == 0), stop=(kc == 1))
                nc.vector.tensor_copy(out=v_ext[0:48, :], in_=v_ps[:])
                x_ps = gps.tile([128, 384], f32, tag="g")
                for k in range(4):
                    m = rr + k
                    for m2 in range(2):
                        sl = slice(k * 96 + m2 * 48, k * 96 + m2 * 48 + 48)
                        for kc in range(2):
                            lo = k * 512 + kc * 256 + m2 * 128
                            nc.tensor.matmul(x_ps[:, sl], lhsT=A1T_s[:, lo:lo + 128],
                                             rhs=st[m][:, kc * 48:kc * 48 + 48],
                                             start=(kc == 0), stop=False)
                            nc.tensor.matmul(x_ps[:, sl], lhsT=A2T_s[:, lo:lo + 128],
                                             rhs=sdeg[k][:, kc * 48:kc * 48 + 48],
                                             start=False, stop=False)
                        nc.tensor.matmul(
                            x_ps[:, sl],
                            lhsT=v_ext[:, k * 256 + m2 * 128:k * 256 + m2 * 128 + 128],
                            rhs=mask_t[m][:], start=False, stop=True)
                a_sb = hb.tile([128, 384], bf16, tag="a")
                nc.scalar.activation(out=a_sb[:], in_=x_ps[:], func=AF.Relu)
                o_ps = gps.tile([128, 384], f32, tag="g")
                for k in range(4):
                    for m2 in range(2):
                        sl = slice(k * 96 + m2 * 48, k * 96 + m2 * 48 + 48)
                        for kc in range(2):
                            lo = k * 512 + kc * 256 + m2 * 128
                            nc.tensor.matmul(o_ps[:, sl], lhsT=gowaT_s[:, lo:lo + 128],
                                             rhs=a_sb[:, k * 96 + kc * 48:
                                                      k * 96 + kc * 48 + 48],
                                             start=(kc == 0), stop=False)
                            nc.tensor.matmul(o_ps[:, sl], lhsT=gowsT_s[:, lo:lo + 128],
                                             rhs=st[rr + k][:, kc * 48:kc * 48 + 48],
                                             start=False, stop=False)
                        nc.tensor.matmul(
                            o_ps[:, sl],
                            lhsT=gob_s[0:1, (k * 2 + m2) * 128:(k * 2 + m2) * 128 + 128],
                            rhs=ones2_s[0:1, 0:48], start=False, stop=True)
                c_sb = hb.tile([128, 384], bf16, tag="c")
                nc.scalar.activation(out=c_sb[:], in_=o_ps[:], func=AF.Relu)
                g_ps = gps.tile([128, 96], f32, tag="gg")
                for m2 in range(2):
                    n = 0
                    for k in range(4):
                        for kc in range(2):
                            lo = (k * 2 + kc) * 256 + m2 * 128
                            nc.tensor.matmul(g_ps[:, m2 * 48:m2 * 48 + 48],
                                             lhsT=aggT_s[:, lo:lo + 128],
                                             rhs=c_sb[:, k * 96 + kc * 48:
                                                      k * 96 + kc * 48 + 48],
                                             start=(n == 0), stop=(n == 7))
                            n += 1
                s_new = st[rr + 4]
                for m2 in range(2):
                    nc.vector.tensor_scalar(out=s_new[:, m2 * 48:m2 * 48 + 48],
                                            in0=g_ps[:, m2 * 48:m2 * 48 + 48],
                                            scalar1=aggb_s[:, m2:m2 + 1], scalar2=None,
                                            op0=OP.add)
                d_ps = sps.tile([4, 48], f32, tag="s")
                for kc in range(2):
                    nc.tensor.matmul(d_ps[:], lhsT=decT_s[:, kc * 4:kc * 4 + 4],
                                     rhs=s_new[:, kc * 48:kc * 48 + 48],
                                     start=(kc == 0), stop=(kc == 1))
                bbv = bbox_sb[:].rearrange("f (b q) -> f b q", b=8)[:, :, rr * 6:rr * 6 + 6]
                nc.vector.tensor_scalar(out=bbv, in0=d_ps[:],
                                        scalar1=decb_s[:, 0:1], scalar2=None, op0=OP.add)
                if rr < 7:
                    m = rr + 4
                    d2_ps = sps.tile([2, 48], f32, tag="s")
                    for kc in range(2):
                        nc.tensor.matmul(d2_ps[:], lhsT=decT_s[:, kc * 4 + 2:kc * 4 + 4],
                                         rhs=s_new[:, kc * 48:kc * 48 + 48],
                                         start=(kc == 0), stop=(kc == 1))
                    coorb = hb.tile([2, 48], bf16, tag="coorb")
                    nc.vector.tensor_scalar(out=coorb[:], in0=d2_ps[:],
                                            scalar1=decb2_s[:, 0:1], scalar2=None, op0=OP.add)
                    cm2 = hb.tile([2, 48], bf16, tag="cm2")
                    nc.vector.tensor_scalar(out=cm2[:], in0=coorb[:], scalar1=-2.0,
                                            scalar2=None, op0=OP.mult)
                    sq = hb.tile([2, 48], bf16, tag="sq")
                    nc.vector.tensor_tensor(out=sq[:], in0=coorb[:], in1=coorb[:], op=OP.mult)
                    m_ps = sps.tile([48, 48], f32, tag="s")
                    nc.tensor.matmul(m_ps[:], lhsT=coorb[:], rhs=cm2[:], start=True, stop=False)
                    nc.tensor.matmul(m_ps[:], lhsT=sq[:], rhs=ones2_s[:], start=False, stop=False)
                    nc.tensor.matmul(m_ps[:], lhsT=ones2_s[:], rhs=sq[:], start=False, stop=True)
                    nc.vector.tensor_tensor(out=mask_t[m][0:48, :], in0=m_ps[:], in1=Tm_s[:],
                                            op=OP.is_le)
                    dd_ps = sps.tile([128, 96], f32, tag="s")
                    for half in range(2):
                        nc.tensor.matmul(dd_ps[:, half * 48:half * 48 + 48],
                                         lhsT=ones48_s[:], rhs=mask_t[m][0:48, :],
                                         start=True, stop=True)
                    nc.vector.tensor_copy(out=deg_t[m][:], in_=dd_ps[:])
                    nc.vector.tensor_copy(out=mask_t[m][64:65, :], in_=dd_ps[0:1, 0:48])
            nc.sync.dma_start(
                out[:].rearrange("b rr n f -> f (b rr n)"), bbox_sb[:])
    return nc


_NC = None

def _get_nc():
    global _NC
    if _NC is None:
        nc = bass.Bass()
        build(nc)
        split_drain_waits(nc)
        _NC = nc
    return _NC


def kernel(**inputs):
    nc = _get_nc()
    inputs = {k: np.asarray(v) for k, v in inputs.items()}
    maps = [make_core_inputs(inputs, s) for s in range(NCORE)]
    res = run_bass_kernel_spmd(nc, maps, core_ids=list(range(NCORE)))
    out = np.concatenate([res.results[s]["bbox_out"] for s in range(NCORE)], 0)
    return out.astype(np.float32)
